# revision 4
# baseline (speedup 1.0000x reference)
"""Trainium2 Bass kernel: batched causal single-head self-attention.

Reference computation (per batch b):
    q = x @ Wq; k = x @ Wk; v = x @ Wv          # [T, H] each, contraction over E
    S = (q @ k^T) / sqrt(H)                     # [T, T]
    P = softmax(causal_mask(S), axis=-1)
    out = P @ v                                 # [T, H]

Shapes: x [512, 256, 384] f32, W* [384, 64] f32, out [512, 256, 64] f32.
Sharding: pure data parallel, 64 batches per NeuronCore across 8 cores.

Device algorithm, quad-granular (4 batches per step), matmuls bf16,
software-pipelined so quad i+1's projections overlap quad i's attention:
  - host ships x^T quad-contiguous ([qd, p, s, c, t]): input DMA = 128
    descriptors x 6KB contiguous (first quad split in two for faster ramp).
  - [k^T; q^T] = [Wk|Wq]^T @ xT  (packed 128-wide stationary, 3 E-chunks,
    512-col moving, one PSUM tile per batch-pair A={0,1}, B={2,3}).
  - k/q staged to SBUF in a PARTITION-PACKED layout for 2-way row-tiled
    scores: kk[0:64]=k(A) / kk[64:128]=k(B), qq[0:64]=q(A) / qq[64:128]=q(B).
    Aligned halves copied on ScalarE, cross-half copies on VectorE (the DVE
    output crossbar supports 64-channel partition-shifted writes).
  - v computed directly in [t, h] layout: stationary = x^T chunk (FWL makes
    the 24 small LDWs cheap), moving = Wv chunk. No PE transposes.  The v
    matmuls are emitted BETWEEN quad q's scores and outs so they fill the
    PE while the exp->mask chain drains.
  - v_aug: persistent SBUF tiles with a ones column at h=64 -> the out
    matmul's column 64 is the softmax denominator for free.
  - scores: K=64 matmuls PACKED 2-per-slot via tile_position=(0,0)/(64,0) --
    batches s and s+2 run CONCURRENTLY in the two 64-row halves of the PE.
  - P    = exp(0.125 * S^T)      (ScalarE, per si-pack; no max-subtraction
    needed, |s| < ~45)
  - P   *= causal 0/1 mask       (diagonal blocks only; per-sp on GpSimdE so
    each gates only its own out matmuls; VectorE near the serial tail)
  - out_aug[tq, 0:65] = P^T V_aug; col 64 = denominator; divided on HOST.
    Output DMA'd in device layout [p, qd, s, j, h] as FP16 (halves output
    HBM traffic; numerator/denominator both fit fp16 comfortably).
  - 6 dummy 512-col matmuls at kernel start trip the PE HAM clock gate
    while the first input DMAs are still landing.
"""

import numpy as np
import ml_dtypes

B, T, E, H = 512, 256, 384, 64
NCORES = 8
BPC = B // NCORES  # 64
P = 128
EC = E // P  # 3
HP1 = H + 1  # 65

_cache: dict = {}


def _install_ntff_hook():
    """Shim antenv.axon_hooks (absent in this image) so run_bass_kernel_spmd
    trace=True can capture NTFF profiles via the axon .so's C ABI."""
    import contextlib
    import ctypes
    import sys
    import types

    if "antenv.axon_hooks" in sys.modules:
        return
    so_path = "/opt/axon/libaxon_pjrt.so"
    lib = ctypes.CDLL(so_path)
    if not hasattr(lib, "axon_start_nrt_profile"):
        return
    lib.axon_start_nrt_profile.argtypes = [
        ctypes.POINTER(ctypes.c_int64),
        ctypes.c_size_t,
    ]
    lib.axon_start_nrt_profile.restype = ctypes.c_int64
    lib.axon_stop_nrt_profile.argtypes = [ctypes.c_char_p]
    lib.axon_stop_nrt_profile.restype = ctypes.c_int64

    @contextlib.contextmanager
    def _hook(output_dir, device_ids):
        import jax

        jax.devices()
        if device_ids:
            ids = (ctypes.c_int64 * len(device_ids))(*device_ids)
            rc = lib.axon_start_nrt_profile(ids, len(device_ids))
        else:
            rc = lib.axon_start_nrt_profile(None, 0)
        if rc != 0:
            raise RuntimeError(f"axon_start_nrt_profile rc={rc}")
        try:
            yield
        finally:
            n = lib.axon_stop_nrt_profile(str(output_dir).encode())
            if n < 0:
                raise RuntimeError(f"axon_stop_nrt_profile rc={n}")
            print(f"profile: {n} file(s) written to {output_dir}", file=sys.stderr)

    mod = types.ModuleType("antenv.axon_hooks")
    _state = {"hook": _hook}
    mod.get_axon_ntff_profile_hook = lambda: _state["hook"]
    mod.set_axon_ntff_profile_hook = lambda h: _state.__setitem__("hook", h)
    sys.modules["antenv.axon_hooks"] = mod


def _build_program(bpc):
    import concourse.bacc as bacc
    import concourse.mybir as mybir
    import concourse.tile as tile

    f32 = mybir.dt.float32
    f16 = mybir.dt.float16
    bf16 = mybir.dt.bfloat16
    Exp = mybir.ActivationFunctionType.Exp
    Mult = mybir.AluOpType.mult

    nc = bacc.Bacc(
        "TRN2",
        target_bir_lowering=False,
        debug=False,
        enable_asserts=False,
        num_devices=NCORES,
    )
    Q = 4
    assert bpc % Q == 0
    nquads = bpc // Q

    xt_d = nc.dram_tensor("xt", [nquads, P, Q, EC, T], bf16, kind="ExternalInput").ap()
    wkq_d = nc.dram_tensor("wkq", [P, EC, P], bf16, kind="ExternalInput").ap()
    wv_d = nc.dram_tensor("wv", [P, EC, H], bf16, kind="ExternalInput").ap()
    um_d = nc.dram_tensor("um", [P, P], bf16, kind="ExternalInput").ap()
    out_d = nc.dram_tensor(
        "out", [P, nquads, Q, 2, HP1], f16, kind="ExternalOutput"
    ).ap()

    with tile.TileContext(nc) as tc:
        with (
            tc.tile_pool(name="const", bufs=1) as constp,
            tc.tile_pool(name="xin", bufs=6) as xpool,
            tc.tile_pool(name="kq", bufs=7) as kqpool,
            tc.tile_pool(name="psb", bufs=3) as ppool,
            tc.tile_pool(name="osb", bufs=3) as opool,
            tc.tile_pool(name="ps_qk", bufs=2, space="PSUM") as ps_qk,
            tc.tile_pool(name="ps_v", bufs=1, space="PSUM") as ps_v,
            tc.tile_pool(name="ps_s", bufs=2, space="PSUM") as ps_s,
            tc.tile_pool(name="ps_o", bufs=1, space="PSUM") as ps_o,
        ):
            # warmup stationary first on the Vector queue so the HAM-warmup
            # matmuls can issue the moment the engines come up
            wrm = constp.tile([P, 2, T], bf16, name="wrm")
            nc.vector.memset(wrm, 0.0)
            # consts + first input quad in two halves (sync-engine DMA issue
            # costs ~650ns each; order = need order)
            wkq = constp.tile([P, EC, P], bf16)
            nc.sync.dma_start(wkq, wkq_d)
            xt0 = xpool.tile([P, Q, EC, T], bf16, name="xt")
            nc.sync.dma_start(xt0[:, 0:2], xt_d[0, :, 0:2])
            nc.sync.dma_start(xt0[:, 2:4], xt_d[0, :, 2:4])
            wv = constp.tile([P, EC, H], bf16)
            nc.sync.dma_start(wv, wv_d)
            um = constp.tile([P, P], bf16)
            nc.sync.dma_start(um, um_d)
            # persistent v_aug tiles (manual double-buffer) with ones column
            vaugs = []
            for i in range(2):
                va = constp.tile([P, Q, 2, HP1], bf16, name=f"vaug{i}")
                nc.vector.memset(va[:, :, :, H : H + 1], 1.0)
                vaugs.append(va)

            # HAM warmup: ~2.5us of cold dummy matmuls (no DMA dependency),
            # sized to end about when the first real operands land.
            wp = ps_s.tile([P, 2, 4, P], f32, name="s_ps")
            for i in range(6):
                nc.tensor.matmul(
                    wp[:, 0, :, :], wrm[:, 0, 0:P], wrm, start=True, stop=True
                )

            def emit_produce_kq(qd):
                """Input DMA + k/q projection + staging for quad qd."""
                if qd == 0:
                    xt = xt0
                else:
                    xt = xpool.tile([P, Q, EC, T], bf16, name="xt")
                    nc.sync.dma_start(xt, xt_d[qd])

                qks = []
                for pr in range(2):
                    s0 = 2 * pr
                    qk_ps = ps_qk.tile([P, 2, T], f32, name="qk_ps")  # 1 bank
                    for c in range(EC):
                        nc.tensor.matmul(
                            qk_ps,
                            wkq[:, c, :],
                            xt[:, s0 : s0 + 2, c, :],
                            start=(c == 0),
                            stop=(c == EC - 1),
                        )
                    qks.append(qk_ps)
                # partition-packed staging for 2-way row-tiled scores:
                # kk[0:64] = k of batches {0,1}, kk[64:128] = k of {2,3};
                # qq[0:64] = q of {0,1},         qq[64:128] = q of {2,3}.
                kk = kqpool.tile([P, 2, T], bf16, name="kk")
                qq = kqpool.tile([P, 2, T], bf16, name="qq")
                nc.vector.tensor_copy(kk[H:P], qks[1][0:H])  # shift +64
                nc.scalar.copy(kk[0:H], qks[0][0:H])  # aligned
                nc.vector.tensor_copy(qq[0:H], qks[0][H:P])  # shift -64
                nc.scalar.copy(qq[H:P], qks[1][H:P])  # aligned
                return xt, kk, qq

            def emit_produce_v(qd, xt):
                """V projection for quad qd (fills the PE while quad qd-1's
                exp->mask chain drains)."""
                v_ps = ps_v.tile([P, Q, 2, H], f32, name="v_ps")  # 1 bank
                for s in range(Q):
                    for j in range(2):
                        for c in range(EC):
                            nc.tensor.matmul(
                                v_ps[:, s, j, :],
                                xt[:, s, c, j * P : (j + 1) * P],
                                wv[:, c, :],
                                start=(c == 0),
                                stop=(c == EC - 1),
                            )
                v_aug = vaugs[qd % 2]
                nc.vector.tensor_copy(v_aug[:, :, :, 0:H], v_ps)
                return v_aug

            def emit_scores(qd, kk, qq, last=False):
                """Packed scores + exp for quad qd."""
                # p_sb dims: [tk-part, sp, tile, blk, tq]; batch s = 2*tile+sp
                p_sb = ppool.tile([P, 2, 2, 3, P], bf16, name="p_sb")
                for sp in range(2):
                    s_ps = ps_s.tile([P, 2, 4, P], f32, name="s_ps")  # 2 banks
                    nc.tensor.matmul(
                        s_ps[:, 0, 0:2, :],
                        kk[0:H, sp, 0:P],
                        qq[0:H, sp, :],
                        start=True,
                        stop=True,
                        tile_position=(0, 0),
                    )
                    nc.tensor.matmul(
                        s_ps[:, 1, 0:2, :],
                        kk[H:P, sp, 0:P],
                        qq[H:P, sp, :],
                        start=True,
                        stop=True,
                        tile_position=(64, 0),
                    )
                    nc.tensor.matmul(
                        s_ps[:, 0, 2, :],
                        kk[0:H, sp, P:T],
                        qq[0:H, sp, P:T],
                        start=True,
                        stop=True,
                        tile_position=(0, 0),
                    )
                    nc.tensor.matmul(
                        s_ps[:, 1, 2, :],
                        kk[H:P, sp, P:T],
                        qq[H:P, sp, P:T],
                        start=True,
                        stop=True,
                        tile_position=(64, 0),
                    )
                    if last:
                        # serial tail: split exp per tile-half for a shorter
                        # dependency chain into the final out matmuls
                        for tl in range(2):
                            nc.scalar.activation(
                                p_sb[:, sp, tl, :, :],
                                s_ps[:, tl, 0:3, :],
                                Exp,
                                scale=0.125,
                            )
                    else:
                        nc.scalar.activation(
                            p_sb[:, sp, :, :, :],
                            s_ps[:, :, 0:3, :],
                            Exp,
                            scale=0.125,
                        )
                return p_sb

            def emit_out(qd, p_sb, v_aug, last=False):
                """Causal mask + out matmuls + output DMA for quad qd."""
                # per-sp masks so each gates only its own out matmuls; the
                # 0::2 stride covers diagonal blocks 0 and 2
                for sp in range(2):
                    eng = nc.vector if last else nc.gpsimd
                    eng.tensor_tensor(
                        p_sb[:, sp, :, 0::2, :],
                        p_sb[:, sp, :, 0::2, :],
                        um[:, None, None, :].to_broadcast([P, 2, 2, P]),
                        Mult,
                    )

                o_sb = opool.tile([P, Q, 2, HP1], f16, name="o_sb")
                for sp in range(2):
                    o_ps = ps_o.tile([P, 2, 2, HP1], f32, name="o_ps")  # 1 bank
                    for tl in range(2):
                        s = 2 * tl + sp
                        nc.tensor.matmul(
                            o_ps[:, tl, 0, :],
                            p_sb[:, sp, tl, 0, :],
                            v_aug[:, s, 0, :],
                            start=True,
                            stop=True,
                        )
                        nc.tensor.matmul(
                            o_ps[:, tl, 1, :],
                            p_sb[:, sp, tl, 1, :],
                            v_aug[:, s, 0, :],
                            start=True,
                            stop=False,
                        )
                        nc.tensor.matmul(
                            o_ps[:, tl, 1, :],
                            p_sb[:, sp, tl, 2, :],
                            v_aug[:, s, 1, :],
                            start=False,
                            stop=True,
                        )
                    # batches of this sp-group are s = sp, sp+2 -> strided dest
                    nc.vector.tensor_copy(o_sb[:, sp :: 2, :, :], o_ps)

                nc.sync.dma_start(out_d[:, qd], o_sb)

            # software pipeline; PE order per iteration:
            #   kq(q+1) -> scores(q) -> v(q+1) -> outs(q)
            xt_c, kk_c, qq_c = emit_produce_kq(0)
            v_aug_c = emit_produce_v(0, xt_c)
            for qd in range(nquads):
                last = qd + 1 == nquads
                tail = qd + 2 >= nquads
                if not last:
                    xt_n, kk_n, qq_n = emit_produce_kq(qd + 1)
                p_sb = emit_scores(qd, kk_c, qq_c, last=last)
                if not last:
                    v_aug_n = emit_produce_v(qd + 1, xt_n)
                emit_out(qd, p_sb, v_aug_c, last=tail)
                if not last:
                    kk_c, qq_c, v_aug_c = kk_n, qq_n, v_aug_n

    nc.compile()
    return nc


def _prep_inputs(x, Wq, Wk, Wv, bpc):
    bf = ml_dtypes.bfloat16
    nb = NCORES * bpc
    nq = bpc // 4
    x = np.asarray(x, dtype=np.float32)[:nb]
    # [b, t, e] -> per core [qd, p, s, c, t] with b = qd*4+s, e = c*128+p
    xt = np.ascontiguousarray(
        x.reshape(NCORES, nq, 4, T, EC, P).transpose(0, 1, 5, 2, 4, 3)
    ).astype(bf)
    wkq = np.concatenate(
        [np.asarray(Wk, np.float32), np.asarray(Wq, np.float32)], axis=1
    )  # [E, 128]: k^T on PSUM partitions 0:64, q^T on 64:128
    wkq = np.ascontiguousarray(wkq.reshape(EC, P, P).transpose(1, 0, 2)).astype(bf)
    wv = np.ascontiguousarray(
        np.asarray(Wv, np.float32).reshape(EC, P, H).transpose(1, 0, 2)
    ).astype(bf)
    tril01 = (np.arange(P)[:, None] <= np.arange(P)[None, :]).astype(np.float32)
    um = tril01.astype(bf)
    per_core = []
    for c in range(NCORES):
        per_core.append(
            {
                "xt": xt[c],
                "wkq": wkq,
                "wv": wv,
                "um": um,
            }
        )
    return per_core


def kernel(x, Wq, Wk, Wv, _trace=False, _bpc=BPC):
    """Full inputs in, full output out. Shards batch dim over 8 NeuronCores."""
    from concourse import bass_utils

    if _trace:
        _install_ntff_hook()

    key = ("prog", _bpc)
    if key not in _cache:
        _cache[key] = _build_program(_bpc)
    nc = _cache[key]

    in_maps = _prep_inputs(x, Wq, Wk, Wv, _bpc)
    res = bass_utils.run_bass_kernel_spmd(
        nc, in_maps, core_ids=list(range(NCORES)), trace=_trace
    )
    _cache["last_result"] = res
    nq = _bpc // 4
    # device layout [p, qd, s, j, h] -> [b, t, h] with b=qd*4+s, t=j*128+p;
    # col 64 is the softmax denominator -> divide here
    outs = []
    for r in res.results:
        o = r["out"].astype(np.float32)
        o = o.reshape(P, nq, 4, 2, HP1).transpose(1, 2, 3, 0, 4)
        o = np.ascontiguousarray(o).reshape(_bpc, T, HP1)
        outs.append(o[:, :, 0:H] / o[:, :, H : H + 1])
    out = np.concatenate(outs, axis=0)
    return out.astype(np.float32)


# revision 5
# speedup vs baseline: 1.0824x; 1.0824x over previous
"""Trainium2 Bass kernel: batched causal single-head self-attention.

Reference computation (per batch b):
    q = x @ Wq; k = x @ Wk; v = x @ Wv          # [T, H] each, contraction over E
    S = (q @ k^T) / sqrt(H)                     # [T, T]
    P = softmax(causal_mask(S), axis=-1)
    out = P @ v                                 # [T, H]

Shapes: x [512, 256, 384] f32, W* [384, 64] f32, out [512, 256, 64] f32.
Sharding: pure data parallel, 64 batches per NeuronCore across 8 cores.

Device algorithm, quad-granular (4 batches per step), matmuls bf16,
software-pipelined so quad i+1's projections overlap quad i's attention:
  - host ships x^T quad-contiguous ([qd, p, s, c, t]): input DMA = 128
    descriptors x 6KB contiguous (first quad split in two for faster ramp).
  - [k^T; q^T] = [Wk|Wq]^T @ xT  (packed 128-wide stationary, 3 E-chunks,
    512-col moving, one PSUM tile per batch-pair A={0,1}, B={2,3}).
  - k/q staged to SBUF in a PARTITION-PACKED layout for 2-way row-tiled
    scores: kk[0:64]=k(A) / kk[64:128]=k(B), qq[0:64]=q(A) / qq[64:128]=q(B).
    Aligned halves copied on ScalarE, cross-half copies on VectorE (the DVE
    output crossbar supports 64-channel partition-shifted writes).
  - v computed directly in [t, h] layout: stationary = x^T chunk (FWL makes
    the 24 small LDWs cheap), moving = Wv chunk. No PE transposes.  The v
    matmuls are emitted BETWEEN quad q's scores and outs so they fill the
    PE while the exp->mask chain drains.
  - v_aug: persistent SBUF tiles with a ones column at h=64 -> the out
    matmul's column 64 is the softmax denominator for free.
  - scores: K=64 matmuls PACKED 2-per-slot via tile_position=(0,0)/(64,0) --
    batches s and s+2 run CONCURRENTLY in the two 64-row halves of the PE.
  - P    = exp(0.125 * S^T)      (ScalarE, per si-pack; no max-subtraction
    needed, |s| < ~45)
  - P   *= causal 0/1 mask       (diagonal blocks only; per-sp on GpSimdE so
    each gates only its own out matmuls; VectorE near the serial tail)
  - out_aug[tq, 0:65] = P^T V_aug; col 64 = denominator; divided on HOST.
    Output DMA'd in device layout [p, qd, s, j, h] as FP16 (halves output
    HBM traffic; numerator/denominator both fit fp16 comfortably).
  - 6 dummy 512-col matmuls at kernel start trip the PE HAM clock gate
    while the first input DMAs are still landing.
"""

import numpy as np
import ml_dtypes

B, T, E, H = 512, 256, 384, 64
NCORES = 8
BPC = B // NCORES  # 64
P = 128
EC = E // P  # 3
HP1 = H + 1  # 65

_cache: dict = {}


def _install_ntff_hook():
    """Shim antenv.axon_hooks (absent in this image) so run_bass_kernel_spmd
    trace=True can capture NTFF profiles via the axon .so's C ABI."""
    import contextlib
    import ctypes
    import sys
    import types

    if "antenv.axon_hooks" in sys.modules:
        return
    so_path = "/opt/axon/libaxon_pjrt.so"
    lib = ctypes.CDLL(so_path)
    if not hasattr(lib, "axon_start_nrt_profile"):
        return
    lib.axon_start_nrt_profile.argtypes = [
        ctypes.POINTER(ctypes.c_int64),
        ctypes.c_size_t,
    ]
    lib.axon_start_nrt_profile.restype = ctypes.c_int64
    lib.axon_stop_nrt_profile.argtypes = [ctypes.c_char_p]
    lib.axon_stop_nrt_profile.restype = ctypes.c_int64

    @contextlib.contextmanager
    def _hook(output_dir, device_ids):
        import jax

        jax.devices()
        if device_ids:
            ids = (ctypes.c_int64 * len(device_ids))(*device_ids)
            rc = lib.axon_start_nrt_profile(ids, len(device_ids))
        else:
            rc = lib.axon_start_nrt_profile(None, 0)
        if rc != 0:
            raise RuntimeError(f"axon_start_nrt_profile rc={rc}")
        try:
            yield
        finally:
            n = lib.axon_stop_nrt_profile(str(output_dir).encode())
            if n < 0:
                raise RuntimeError(f"axon_stop_nrt_profile rc={n}")
            print(f"profile: {n} file(s) written to {output_dir}", file=sys.stderr)

    mod = types.ModuleType("antenv.axon_hooks")
    _state = {"hook": _hook}
    mod.get_axon_ntff_profile_hook = lambda: _state["hook"]
    mod.set_axon_ntff_profile_hook = lambda h: _state.__setitem__("hook", h)
    sys.modules["antenv.axon_hooks"] = mod


def _build_program(bpc):
    import concourse.bacc as bacc
    import concourse.mybir as mybir
    import concourse.tile as tile

    f32 = mybir.dt.float32
    f16 = mybir.dt.float16
    bf16 = mybir.dt.bfloat16
    Exp = mybir.ActivationFunctionType.Exp
    Mult = mybir.AluOpType.mult

    nc = bacc.Bacc(
        "TRN2",
        target_bir_lowering=False,
        debug=False,
        enable_asserts=False,
        num_devices=NCORES,
    )
    Q = 4
    assert bpc % Q == 0
    nquads = bpc // Q

    xt_d = nc.dram_tensor("xt", [nquads, P, Q, EC, T], bf16, kind="ExternalInput").ap()
    wkq_d = nc.dram_tensor("wkq", [P, EC, P], bf16, kind="ExternalInput").ap()
    wv_d = nc.dram_tensor("wv", [P, EC, H], bf16, kind="ExternalInput").ap()
    um_d = nc.dram_tensor("um", [P, P], bf16, kind="ExternalInput").ap()
    out_d = nc.dram_tensor(
        "out", [P, nquads, Q, 2, HP1], f16, kind="ExternalOutput"
    ).ap()

    with tile.TileContext(nc) as tc:
        with (
            tc.tile_pool(name="const", bufs=1) as constp,
            tc.tile_pool(name="xin", bufs=6) as xpool,
            tc.tile_pool(name="kq", bufs=7) as kqpool,
            tc.tile_pool(name="psb", bufs=3) as ppool,
            tc.tile_pool(name="osb", bufs=3) as opool,
            tc.tile_pool(name="ps_qk", bufs=2, space="PSUM") as ps_qk,
            tc.tile_pool(name="ps_v", bufs=1, space="PSUM") as ps_v,
            tc.tile_pool(name="ps_s", bufs=2, space="PSUM") as ps_s,
            tc.tile_pool(name="ps_o", bufs=1, space="PSUM") as ps_o,
        ):
            # warmup stationary first on the Vector queue so the HAM-warmup
            # matmuls can issue the moment the engines come up
            wrm = constp.tile([P, 2, T], bf16, name="wrm")
            nc.vector.memset(wrm, 0.0)
            # consts + first input quad in two halves (sync-engine DMA issue
            # costs ~650ns each; order = need order)
            wkq = constp.tile([P, EC, P], bf16)
            nc.sync.dma_start(wkq, wkq_d)
            xt0 = xpool.tile([P, Q, EC, T], bf16, name="xt")
            nc.sync.dma_start(xt0[:, 0:2], xt_d[0, :, 0:2])
            nc.sync.dma_start(xt0[:, 2:4], xt_d[0, :, 2:4])
            wv = constp.tile([P, EC, H], bf16)
            nc.sync.dma_start(wv, wv_d)
            um = constp.tile([P, P], bf16)
            nc.sync.dma_start(um, um_d)
            # persistent v_aug tiles (manual double-buffer) with ones column
            vaugs = []
            for i in range(2):
                va = constp.tile([P, Q, 2, HP1], bf16, name=f"vaug{i}")
                nc.vector.memset(va[:, :, :, H : H + 1], 1.0)
                vaugs.append(va)

            # HAM warmup: ~2.5us of cold dummy matmuls (no DMA dependency),
            # sized to end about when the first real operands land.
            wp = ps_s.tile([P, 2, 4, P], f32, name="s_ps")
            for i in range(6):
                nc.tensor.matmul(
                    wp[:, 0, :, :], wrm[:, 0, 0:P], wrm, start=True, stop=True
                )

            def emit_produce_kq(qd):
                """Input DMA + k/q projection + staging for quad qd."""
                if qd == 0:
                    xt = xt0
                else:
                    xt = xpool.tile([P, Q, EC, T], bf16, name="xt")
                    nc.sync.dma_start(xt, xt_d[qd])

                qks = []
                for pr in range(2):
                    s0 = 2 * pr
                    qk_ps = ps_qk.tile([P, 2, T], f32, name="qk_ps")  # 1 bank
                    for c in range(EC):
                        nc.tensor.matmul(
                            qk_ps,
                            wkq[:, c, :],
                            xt[:, s0 : s0 + 2, c, :],
                            start=(c == 0),
                            stop=(c == EC - 1),
                        )
                    qks.append(qk_ps)
                # partition-packed staging for 2-way row-tiled scores:
                # kk[0:64] = k of batches {0,1}, kk[64:128] = k of {2,3};
                # qq[0:64] = q of {0,1},         qq[64:128] = q of {2,3}.
                kk = kqpool.tile([P, 2, T], bf16, name="kk")
                qq = kqpool.tile([P, 2, T], bf16, name="qq")
                nc.vector.tensor_copy(kk[H:P], qks[1][0:H])  # shift +64
                nc.scalar.copy(kk[0:H], qks[0][0:H])  # aligned
                nc.vector.tensor_copy(qq[0:H], qks[0][H:P])  # shift -64
                nc.scalar.copy(qq[H:P], qks[1][H:P])  # aligned
                return xt, kk, qq

            def emit_produce_v(qd, xt):
                """V projection for quad qd (fills the PE while quad qd-1's
                exp->mask chain drains)."""
                v_ps = ps_v.tile([P, Q, 2, H], f32, name="v_ps")  # 1 bank
                for s in range(Q):
                    for j in range(2):
                        for c in range(EC):
                            nc.tensor.matmul(
                                v_ps[:, s, j, :],
                                xt[:, s, c, j * P : (j + 1) * P],
                                wv[:, c, :],
                                start=(c == 0),
                                stop=(c == EC - 1),
                            )
                v_aug = vaugs[qd % 2]
                nc.vector.tensor_copy(v_aug[:, :, :, 0:H], v_ps)
                return v_aug

            def emit_scores(qd, kk, qq, last=False):
                """Packed scores + exp for quad qd."""
                # p_sb dims: [tk-part, sp, tile, blk, tq]; batch s = 2*tile+sp
                p_sb = ppool.tile([P, 2, 2, 3, P], bf16, name="p_sb")
                for sp in range(2):
                    s_ps = ps_s.tile([P, 2, 4, P], f32, name="s_ps")  # 2 banks
                    nc.tensor.matmul(
                        s_ps[:, 0, 0:2, :],
                        kk[0:H, sp, 0:P],
                        qq[0:H, sp, :],
                        start=True,
                        stop=True,
                        tile_position=(0, 0),
                    )
                    nc.tensor.matmul(
                        s_ps[:, 1, 0:2, :],
                        kk[H:P, sp, 0:P],
                        qq[H:P, sp, :],
                        start=True,
                        stop=True,
                        tile_position=(64, 0),
                    )
                    nc.tensor.matmul(
                        s_ps[:, 0, 2, :],
                        kk[0:H, sp, P:T],
                        qq[0:H, sp, P:T],
                        start=True,
                        stop=True,
                        tile_position=(0, 0),
                    )
                    nc.tensor.matmul(
                        s_ps[:, 1, 2, :],
                        kk[H:P, sp, P:T],
                        qq[H:P, sp, P:T],
                        start=True,
                        stop=True,
                        tile_position=(64, 0),
                    )
                    if last:
                        # serial tail: split exp per tile-half for a shorter
                        # dependency chain into the final out matmuls
                        for tl in range(2):
                            nc.scalar.activation(
                                p_sb[:, sp, tl, :, :],
                                s_ps[:, tl, 0:3, :],
                                Exp,
                                scale=0.125,
                            )
                    else:
                        nc.scalar.activation(
                            p_sb[:, sp, :, :, :],
                            s_ps[:, :, 0:3, :],
                            Exp,
                            scale=0.125,
                        )
                return p_sb

            def emit_out(qd, p_sb, v_aug, last=False):
                """Causal mask + out matmuls + output DMA for quad qd."""
                # per-sp masks so each gates only its own out matmuls; the
                # 0::2 stride covers diagonal blocks 0 and 2
                for sp in range(2):
                    eng = nc.vector if last else nc.gpsimd
                    eng.tensor_tensor(
                        p_sb[:, sp, :, 0::2, :],
                        p_sb[:, sp, :, 0::2, :],
                        um[:, None, None, :].to_broadcast([P, 2, 2, P]),
                        Mult,
                    )

                o_sb = opool.tile([P, Q, 2, HP1], f16, name="o_sb")
                for sp in range(2):
                    o_ps = ps_o.tile([P, 2, 2, HP1], f32, name="o_ps")  # 1 bank
                    for tl in range(2):
                        s = 2 * tl + sp
                        nc.tensor.matmul(
                            o_ps[:, tl, 0, :],
                            p_sb[:, sp, tl, 0, :],
                            v_aug[:, s, 0, :],
                            start=True,
                            stop=True,
                        )
                        nc.tensor.matmul(
                            o_ps[:, tl, 1, :],
                            p_sb[:, sp, tl, 1, :],
                            v_aug[:, s, 0, :],
                            start=True,
                            stop=False,
                        )
                        nc.tensor.matmul(
                            o_ps[:, tl, 1, :],
                            p_sb[:, sp, tl, 2, :],
                            v_aug[:, s, 1, :],
                            start=False,
                            stop=True,
                        )
                    # batches of this sp-group are s = sp, sp+2 -> strided dest
                    nc.vector.tensor_copy(o_sb[:, sp :: 2, :, :], o_ps)

                nc.sync.dma_start(out_d[:, qd], o_sb)

            # software pipeline: produce runs one quad ahead of consume (the
            # Tile scheduler interleaves produce(q+1) into consume(q)'s
            # exp->mask latency on its own)
            xt_c, kk_c, qq_c = emit_produce_kq(0)
            v_aug_c = emit_produce_v(0, xt_c)
            for qd in range(nquads):
                last = qd + 1 == nquads
                tail = qd + 2 >= nquads
                if not last:
                    xt_n, kk_n, qq_n = emit_produce_kq(qd + 1)
                    v_aug_n = emit_produce_v(qd + 1, xt_n)
                p_sb = emit_scores(qd, kk_c, qq_c, last=last)
                emit_out(qd, p_sb, v_aug_c, last=tail)
                if not last:
                    kk_c, qq_c, v_aug_c = kk_n, qq_n, v_aug_n

    nc.compile()
    return nc


def _prep_inputs(x, Wq, Wk, Wv, bpc):
    bf = ml_dtypes.bfloat16
    nb = NCORES * bpc
    nq = bpc // 4
    x = np.asarray(x, dtype=np.float32)[:nb]
    # [b, t, e] -> per core [qd, p, s, c, t] with b = qd*4+s, e = c*128+p
    xt = np.ascontiguousarray(
        x.reshape(NCORES, nq, 4, T, EC, P).transpose(0, 1, 5, 2, 4, 3)
    ).astype(bf)
    wkq = np.concatenate(
        [np.asarray(Wk, np.float32), np.asarray(Wq, np.float32)], axis=1
    )  # [E, 128]: k^T on PSUM partitions 0:64, q^T on 64:128
    wkq = np.ascontiguousarray(wkq.reshape(EC, P, P).transpose(1, 0, 2)).astype(bf)
    wv = np.ascontiguousarray(
        np.asarray(Wv, np.float32).reshape(EC, P, H).transpose(1, 0, 2)
    ).astype(bf)
    tril01 = (np.arange(P)[:, None] <= np.arange(P)[None, :]).astype(np.float32)
    um = tril01.astype(bf)
    per_core = []
    for c in range(NCORES):
        per_core.append(
            {
                "xt": xt[c],
                "wkq": wkq,
                "wv": wv,
                "um": um,
            }
        )
    return per_core


def kernel(x, Wq, Wk, Wv, _trace=False, _bpc=BPC):
    """Full inputs in, full output out. Shards batch dim over 8 NeuronCores."""
    from concourse import bass_utils

    if _trace:
        _install_ntff_hook()

    key = ("prog", _bpc)
    if key not in _cache:
        _cache[key] = _build_program(_bpc)
    nc = _cache[key]

    in_maps = _prep_inputs(x, Wq, Wk, Wv, _bpc)
    res = bass_utils.run_bass_kernel_spmd(
        nc, in_maps, core_ids=list(range(NCORES)), trace=_trace
    )
    _cache["last_result"] = res
    nq = _bpc // 4
    # device layout [p, qd, s, j, h] -> [b, t, h] with b=qd*4+s, t=j*128+p;
    # col 64 is the softmax denominator -> divide here
    outs = []
    for r in res.results:
        o = r["out"].astype(np.float32)
        o = o.reshape(P, nq, 4, 2, HP1).transpose(1, 2, 3, 0, 4)
        o = np.ascontiguousarray(o).reshape(_bpc, T, HP1)
        outs.append(o[:, :, 0:H] / o[:, :, H : H + 1])
    out = np.concatenate(outs, axis=0)
    return out.astype(np.float32)


# revision 6
# speedup vs baseline: 1.0857x; 1.0031x over previous
"""Trainium2 Bass kernel: batched causal single-head self-attention.

Reference computation (per batch b):
    q = x @ Wq; k = x @ Wk; v = x @ Wv          # [T, H] each, contraction over E
    S = (q @ k^T) / sqrt(H)                     # [T, T]
    P = softmax(causal_mask(S), axis=-1)
    out = P @ v                                 # [T, H]

Shapes: x [512, 256, 384] f32, W* [384, 64] f32, out [512, 256, 64] f32.
Sharding: pure data parallel, 64 batches per NeuronCore across 8 cores.

Device algorithm, quad-granular (4 batches per step), matmuls bf16,
software-pipelined so quad i+1's projections overlap quad i's attention:
  - host ships x^T quad-contiguous ([qd, p, s, c, t]): input DMA = 128
    descriptors x 6KB contiguous (first quad split in two for faster ramp).
  - [k^T; q^T] = [Wk|Wq]^T @ xT  (packed 128-wide stationary, 3 E-chunks,
    512-col moving, one PSUM tile per batch-pair A={0,1}, B={2,3}).
  - k/q staged to SBUF in a PARTITION-PACKED layout for 2-way row-tiled
    scores: kk[0:64]=k(A) / kk[64:128]=k(B), qq[0:64]=q(A) / qq[64:128]=q(B).
    Aligned halves copied on ScalarE, cross-half copies on VectorE (the DVE
    output crossbar supports 64-channel partition-shifted writes).
  - v computed directly in [t, h] layout: stationary = x^T chunk (FWL makes
    the 24 small LDWs cheap), moving = Wv chunk. No PE transposes.  The v
    matmuls are emitted BETWEEN quad q's scores and outs so they fill the
    PE while the exp->mask chain drains.
  - v_aug: persistent SBUF tiles with a ones column at h=64 -> the out
    matmul's column 64 is the softmax denominator for free.
  - scores: K=64 matmuls PACKED 2-per-slot via tile_position=(0,0)/(64,0) --
    batches s and s+2 run CONCURRENTLY in the two 64-row halves of the PE.
  - P    = exp(0.125 * S^T)      (ScalarE, per si-pack; no max-subtraction
    needed, |s| < ~45)
  - P   *= causal 0/1 mask       (diagonal blocks only; per-sp on GpSimdE so
    each gates only its own out matmuls; VectorE near the serial tail)
  - out_aug[tq, 0:65] = P^T V_aug; col 64 = denominator; divided on HOST.
    Output DMA'd in device layout [p, qd, s, j, h] as FP16 (halves output
    HBM traffic; numerator/denominator both fit fp16 comfortably).
  - 6 dummy 512-col matmuls at kernel start trip the PE HAM clock gate
    while the first input DMAs are still landing.
"""

import numpy as np
import ml_dtypes

B, T, E, H = 512, 256, 384, 64
NCORES = 8
BPC = B // NCORES  # 64
P = 128
EC = E // P  # 3
HP1 = H + 1  # 65

_cache: dict = {}


def _install_ntff_hook():
    """Shim antenv.axon_hooks (absent in this image) so run_bass_kernel_spmd
    trace=True can capture NTFF profiles via the axon .so's C ABI."""
    import contextlib
    import ctypes
    import sys
    import types

    if "antenv.axon_hooks" in sys.modules:
        return
    so_path = "/opt/axon/libaxon_pjrt.so"
    lib = ctypes.CDLL(so_path)
    if not hasattr(lib, "axon_start_nrt_profile"):
        return
    lib.axon_start_nrt_profile.argtypes = [
        ctypes.POINTER(ctypes.c_int64),
        ctypes.c_size_t,
    ]
    lib.axon_start_nrt_profile.restype = ctypes.c_int64
    lib.axon_stop_nrt_profile.argtypes = [ctypes.c_char_p]
    lib.axon_stop_nrt_profile.restype = ctypes.c_int64

    @contextlib.contextmanager
    def _hook(output_dir, device_ids):
        import jax

        jax.devices()
        if device_ids:
            ids = (ctypes.c_int64 * len(device_ids))(*device_ids)
            rc = lib.axon_start_nrt_profile(ids, len(device_ids))
        else:
            rc = lib.axon_start_nrt_profile(None, 0)
        if rc != 0:
            raise RuntimeError(f"axon_start_nrt_profile rc={rc}")
        try:
            yield
        finally:
            n = lib.axon_stop_nrt_profile(str(output_dir).encode())
            if n < 0:
                raise RuntimeError(f"axon_stop_nrt_profile rc={n}")
            print(f"profile: {n} file(s) written to {output_dir}", file=sys.stderr)

    mod = types.ModuleType("antenv.axon_hooks")
    _state = {"hook": _hook}
    mod.get_axon_ntff_profile_hook = lambda: _state["hook"]
    mod.set_axon_ntff_profile_hook = lambda h: _state.__setitem__("hook", h)
    sys.modules["antenv.axon_hooks"] = mod


def _build_program(bpc):
    import concourse.bacc as bacc
    import concourse.mybir as mybir
    import concourse.tile as tile

    f32 = mybir.dt.float32
    f16 = mybir.dt.float16
    bf16 = mybir.dt.bfloat16
    Exp = mybir.ActivationFunctionType.Exp
    Mult = mybir.AluOpType.mult

    nc = bacc.Bacc(
        "TRN2",
        target_bir_lowering=False,
        debug=False,
        enable_asserts=False,
        num_devices=NCORES,
    )
    Q = 4
    assert bpc % Q == 0
    nquads = bpc // Q

    xt_d = nc.dram_tensor("xt", [nquads, P, Q, EC, T], bf16, kind="ExternalInput").ap()
    wkq_d = nc.dram_tensor("wkq", [P, EC, P], bf16, kind="ExternalInput").ap()
    wv_d = nc.dram_tensor("wv", [P, EC, H], bf16, kind="ExternalInput").ap()
    um_d = nc.dram_tensor("um", [P, P], bf16, kind="ExternalInput").ap()
    out_d = nc.dram_tensor(
        "out", [P, nquads, Q, 2, HP1], f16, kind="ExternalOutput"
    ).ap()

    with tile.TileContext(nc) as tc:
        with (
            tc.tile_pool(name="const", bufs=1) as constp,
            tc.tile_pool(name="xin", bufs=6) as xpool,
            tc.tile_pool(name="kq", bufs=7) as kqpool,
            tc.tile_pool(name="psb", bufs=3) as ppool,
            tc.tile_pool(name="osb", bufs=3) as opool,
            tc.tile_pool(name="ps_qk", bufs=2, space="PSUM") as ps_qk,
            tc.tile_pool(name="ps_v", bufs=1, space="PSUM") as ps_v,
            tc.tile_pool(name="ps_s", bufs=2, space="PSUM") as ps_s,
            tc.tile_pool(name="ps_o", bufs=1, space="PSUM") as ps_o,
        ):
            # warmup stationary first on the Vector queue so the HAM-warmup
            # matmuls can issue the moment the engines come up
            wrm = constp.tile([P, 2, T], bf16, name="wrm")
            nc.vector.memset(wrm, 0.0)
            # consts + first input quad in two halves (sync-engine DMA issue
            # costs ~650ns each; order = need order)
            wkq = constp.tile([P, EC, P], bf16)
            nc.sync.dma_start(wkq, wkq_d)
            xt0 = xpool.tile([P, Q, EC, T], bf16, name="xt")
            nc.sync.dma_start(xt0[:, 0:2], xt_d[0, :, 0:2])
            nc.sync.dma_start(xt0[:, 2:4], xt_d[0, :, 2:4])
            wv = constp.tile([P, EC, H], bf16)
            nc.sync.dma_start(wv, wv_d)
            um = constp.tile([P, P], bf16)
            nc.sync.dma_start(um, um_d)
            # persistent v_aug tiles (manual double-buffer) with ones column
            vaugs = []
            for i in range(2):
                va = constp.tile([P, Q, 2, HP1], bf16, name=f"vaug{i}")
                nc.vector.memset(va[:, :, :, H : H + 1], 1.0)
                vaugs.append(va)

            # HAM warmup: ~4us of cold dummy matmuls (no DMA dependency) --
            # must exceed the 3.4us HAM activity window to un-throttle the
            # PE clock before real work arrives.
            wp = ps_s.tile([P, 2, 4, P], f32, name="s_ps")
            for i in range(10):
                nc.tensor.matmul(
                    wp[:, 0, :, :], wrm[:, 0, 0:P], wrm, start=True, stop=True
                )

            def emit_produce_kq(qd):
                """Input DMA + k/q projection + staging for quad qd."""
                if qd == 0:
                    xt = xt0
                else:
                    xt = xpool.tile([P, Q, EC, T], bf16, name="xt")
                    nc.sync.dma_start(xt, xt_d[qd])

                qks = []
                for pr in range(2):
                    s0 = 2 * pr
                    qk_ps = ps_qk.tile([P, 2, T], f32, name="qk_ps")  # 1 bank
                    for c in range(EC):
                        nc.tensor.matmul(
                            qk_ps,
                            wkq[:, c, :],
                            xt[:, s0 : s0 + 2, c, :],
                            start=(c == 0),
                            stop=(c == EC - 1),
                        )
                    qks.append(qk_ps)
                # partition-packed staging for 2-way row-tiled scores:
                # kk[0:64] = k of batches {0,1}, kk[64:128] = k of {2,3};
                # qq[0:64] = q of {0,1},         qq[64:128] = q of {2,3}.
                kk = kqpool.tile([P, 2, T], bf16, name="kk")
                qq = kqpool.tile([P, 2, T], bf16, name="qq")
                nc.vector.tensor_copy(kk[H:P], qks[1][0:H])  # shift +64
                nc.scalar.copy(kk[0:H], qks[0][0:H])  # aligned
                nc.vector.tensor_copy(qq[0:H], qks[0][H:P])  # shift -64
                nc.scalar.copy(qq[H:P], qks[1][H:P])  # aligned
                return xt, kk, qq

            def emit_produce_v(qd, xt):
                """V projection for quad qd (fills the PE while quad qd-1's
                exp->mask chain drains)."""
                v_ps = ps_v.tile([P, Q, 2, H], f32, name="v_ps")  # 1 bank
                for s in range(Q):
                    for j in range(2):
                        for c in range(EC):
                            nc.tensor.matmul(
                                v_ps[:, s, j, :],
                                xt[:, s, c, j * P : (j + 1) * P],
                                wv[:, c, :],
                                start=(c == 0),
                                stop=(c == EC - 1),
                            )
                v_aug = vaugs[qd % 2]
                nc.vector.tensor_copy(v_aug[:, :, :, 0:H], v_ps)
                return v_aug

            def emit_scores(qd, kk, qq, last=False):
                """Packed scores + exp for quad qd."""
                # p_sb dims: [tk-part, sp, tile, blk, tq]; batch s = 2*tile+sp
                p_sb = ppool.tile([P, 2, 2, 3, P], bf16, name="p_sb")
                for sp in range(2):
                    s_ps = ps_s.tile([P, 2, 4, P], f32, name="s_ps")  # 2 banks
                    nc.tensor.matmul(
                        s_ps[:, 0, 0:2, :],
                        kk[0:H, sp, 0:P],
                        qq[0:H, sp, :],
                        start=True,
                        stop=True,
                        tile_position=(0, 0),
                    )
                    nc.tensor.matmul(
                        s_ps[:, 1, 0:2, :],
                        kk[H:P, sp, 0:P],
                        qq[H:P, sp, :],
                        start=True,
                        stop=True,
                        tile_position=(64, 0),
                    )
                    nc.tensor.matmul(
                        s_ps[:, 0, 2, :],
                        kk[0:H, sp, P:T],
                        qq[0:H, sp, P:T],
                        start=True,
                        stop=True,
                        tile_position=(0, 0),
                    )
                    nc.tensor.matmul(
                        s_ps[:, 1, 2, :],
                        kk[H:P, sp, P:T],
                        qq[H:P, sp, P:T],
                        start=True,
                        stop=True,
                        tile_position=(64, 0),
                    )
                    if last:
                        # serial tail: split exp per tile-half for a shorter
                        # dependency chain into the final out matmuls
                        for tl in range(2):
                            nc.scalar.activation(
                                p_sb[:, sp, tl, :, :],
                                s_ps[:, tl, 0:3, :],
                                Exp,
                                scale=0.125,
                            )
                    else:
                        nc.scalar.activation(
                            p_sb[:, sp, :, :, :],
                            s_ps[:, :, 0:3, :],
                            Exp,
                            scale=0.125,
                        )
                return p_sb

            def emit_out(qd, p_sb, v_aug, last=False):
                """Causal mask + out matmuls + output DMA for quad qd."""
                # per-sp masks so each gates only its own out matmuls; the
                # 0::2 stride covers diagonal blocks 0 and 2
                for sp in range(2):
                    eng = nc.vector if last else nc.gpsimd
                    eng.tensor_tensor(
                        p_sb[:, sp, :, 0::2, :],
                        p_sb[:, sp, :, 0::2, :],
                        um[:, None, None, :].to_broadcast([P, 2, 2, P]),
                        Mult,
                    )

                o_sb = opool.tile([P, Q, 2, HP1], f16, name="o_sb")
                for sp in range(2):
                    o_ps = ps_o.tile([P, 2, 2, HP1], f32, name="o_ps")  # 1 bank
                    for tl in range(2):
                        s = 2 * tl + sp
                        nc.tensor.matmul(
                            o_ps[:, tl, 0, :],
                            p_sb[:, sp, tl, 0, :],
                            v_aug[:, s, 0, :],
                            start=True,
                            stop=True,
                        )
                        nc.tensor.matmul(
                            o_ps[:, tl, 1, :],
                            p_sb[:, sp, tl, 1, :],
                            v_aug[:, s, 0, :],
                            start=True,
                            stop=False,
                        )
                        nc.tensor.matmul(
                            o_ps[:, tl, 1, :],
                            p_sb[:, sp, tl, 2, :],
                            v_aug[:, s, 1, :],
                            start=False,
                            stop=True,
                        )
                    # batches of this sp-group are s = sp, sp+2 -> strided dest
                    nc.vector.tensor_copy(o_sb[:, sp :: 2, :, :], o_ps)

                nc.sync.dma_start(out_d[:, qd], o_sb)

            # software pipeline: produce runs one quad ahead of consume (the
            # Tile scheduler interleaves produce(q+1) into consume(q)'s
            # exp->mask latency on its own)
            xt_c, kk_c, qq_c = emit_produce_kq(0)
            v_aug_c = emit_produce_v(0, xt_c)
            for qd in range(nquads):
                last = qd + 1 == nquads
                tail = qd + 2 >= nquads
                if not last:
                    xt_n, kk_n, qq_n = emit_produce_kq(qd + 1)
                    v_aug_n = emit_produce_v(qd + 1, xt_n)
                p_sb = emit_scores(qd, kk_c, qq_c, last=last)
                emit_out(qd, p_sb, v_aug_c, last=tail)
                if not last:
                    kk_c, qq_c, v_aug_c = kk_n, qq_n, v_aug_n

    nc.compile()
    return nc


def _prep_inputs(x, Wq, Wk, Wv, bpc):
    bf = ml_dtypes.bfloat16
    nb = NCORES * bpc
    nq = bpc // 4
    x = np.asarray(x, dtype=np.float32)[:nb]
    # [b, t, e] -> per core [qd, p, s, c, t] with b = qd*4+s, e = c*128+p
    xt = np.ascontiguousarray(
        x.reshape(NCORES, nq, 4, T, EC, P).transpose(0, 1, 5, 2, 4, 3)
    ).astype(bf)
    wkq = np.concatenate(
        [np.asarray(Wk, np.float32), np.asarray(Wq, np.float32)], axis=1
    )  # [E, 128]: k^T on PSUM partitions 0:64, q^T on 64:128
    wkq = np.ascontiguousarray(wkq.reshape(EC, P, P).transpose(1, 0, 2)).astype(bf)
    wv = np.ascontiguousarray(
        np.asarray(Wv, np.float32).reshape(EC, P, H).transpose(1, 0, 2)
    ).astype(bf)
    tril01 = (np.arange(P)[:, None] <= np.arange(P)[None, :]).astype(np.float32)
    um = tril01.astype(bf)
    per_core = []
    for c in range(NCORES):
        per_core.append(
            {
                "xt": xt[c],
                "wkq": wkq,
                "wv": wv,
                "um": um,
            }
        )
    return per_core


def kernel(x, Wq, Wk, Wv, _trace=False, _bpc=BPC):
    """Full inputs in, full output out. Shards batch dim over 8 NeuronCores."""
    from concourse import bass_utils

    if _trace:
        _install_ntff_hook()

    key = ("prog", _bpc)
    if key not in _cache:
        _cache[key] = _build_program(_bpc)
    nc = _cache[key]

    in_maps = _prep_inputs(x, Wq, Wk, Wv, _bpc)
    res = bass_utils.run_bass_kernel_spmd(
        nc, in_maps, core_ids=list(range(NCORES)), trace=_trace
    )
    _cache["last_result"] = res
    nq = _bpc // 4
    # device layout [p, qd, s, j, h] -> [b, t, h] with b=qd*4+s, t=j*128+p;
    # col 64 is the softmax denominator -> divide here
    outs = []
    for r in res.results:
        o = r["out"].astype(np.float32)
        o = o.reshape(P, nq, 4, 2, HP1).transpose(1, 2, 3, 0, 4)
        o = np.ascontiguousarray(o).reshape(_bpc, T, HP1)
        outs.append(o[:, :, 0:H] / o[:, :, H : H + 1])
    out = np.concatenate(outs, axis=0)
    return out.astype(np.float32)


# revision 7
# speedup vs baseline: 1.1217x; 1.0331x over previous
"""Trainium2 Bass kernel: batched causal single-head self-attention.

Reference computation (per batch b):
    q = x @ Wq; k = x @ Wk; v = x @ Wv          # [T, H] each, contraction over E
    S = (q @ k^T) / sqrt(H)                     # [T, T]
    P = softmax(causal_mask(S), axis=-1)
    out = P @ v                                 # [T, H]

Shapes: x [512, 256, 384] f32, W* [384, 64] f32, out [512, 256, 64] f32.
Sharding: pure data parallel, 64 batches per NeuronCore across 8 cores.

Device algorithm, quad-granular (4 batches per step), matmuls bf16,
software-pipelined so quad i+1's projections overlap quad i's attention:
  - host ships x^T quad-contiguous ([qd, p, s, c, t]): input DMA = 128
    descriptors x 6KB contiguous (first quad split in two for faster ramp).
  - [k^T; q^T] = [Wk|Wq]^T @ xT  (packed 128-wide stationary, 3 E-chunks,
    512-col moving, one PSUM tile per batch-pair A={0,1}, B={2,3}).
  - k/q staged to SBUF in a PARTITION-PACKED layout for 2-way row-tiled
    scores: kk[0:64]=k(A) / kk[64:128]=k(B), qq[0:64]=q(A) / qq[64:128]=q(B).
    Aligned halves copied on ScalarE, cross-half copies on VectorE (the DVE
    output crossbar supports 64-channel partition-shifted writes).
  - v computed directly in [t, h] layout: stationary = x^T chunk (FWL makes
    the 24 small LDWs cheap), moving = Wv chunk. No PE transposes.  The v
    matmuls are emitted BETWEEN quad q's scores and outs so they fill the
    PE while the exp->mask chain drains.
  - v_aug: persistent SBUF tiles with a ones column at h=64 -> the out
    matmul's column 64 is the softmax denominator for free.
  - scores: K=64 matmuls PACKED 2-per-slot via tile_position=(0,0)/(64,0) --
    batches s and s+2 run CONCURRENTLY in the two 64-row halves of the PE.
  - P    = exp(0.125 * S^T)      (ScalarE, per si-pack; no max-subtraction
    needed, |s| < ~45)
  - P   *= causal 0/1 mask       (diagonal blocks only; per-sp on GpSimdE so
    each gates only its own out matmuls; VectorE near the serial tail)
  - out_aug[tq, 0:65] = P^T V_aug; col 64 = denominator; divided on HOST.
    Output DMA'd in device layout [p, qd, s, j, h] as FP16 (halves output
    HBM traffic; numerator/denominator both fit fp16 comfortably).
  - 6 dummy 512-col matmuls at kernel start trip the PE HAM clock gate
    while the first input DMAs are still landing.
"""

import numpy as np
import ml_dtypes

B, T, E, H = 512, 256, 384, 64
NCORES = 8
BPC = B // NCORES  # 64
P = 128
EC = E // P  # 3
HP1 = H + 1  # 65

_cache: dict = {}


def _install_ntff_hook():
    """Shim antenv.axon_hooks (absent in this image) so run_bass_kernel_spmd
    trace=True can capture NTFF profiles via the axon .so's C ABI."""
    import contextlib
    import ctypes
    import sys
    import types

    if "antenv.axon_hooks" in sys.modules:
        return
    so_path = "/opt/axon/libaxon_pjrt.so"
    lib = ctypes.CDLL(so_path)
    if not hasattr(lib, "axon_start_nrt_profile"):
        return
    lib.axon_start_nrt_profile.argtypes = [
        ctypes.POINTER(ctypes.c_int64),
        ctypes.c_size_t,
    ]
    lib.axon_start_nrt_profile.restype = ctypes.c_int64
    lib.axon_stop_nrt_profile.argtypes = [ctypes.c_char_p]
    lib.axon_stop_nrt_profile.restype = ctypes.c_int64

    @contextlib.contextmanager
    def _hook(output_dir, device_ids):
        import jax

        jax.devices()
        if device_ids:
            ids = (ctypes.c_int64 * len(device_ids))(*device_ids)
            rc = lib.axon_start_nrt_profile(ids, len(device_ids))
        else:
            rc = lib.axon_start_nrt_profile(None, 0)
        if rc != 0:
            raise RuntimeError(f"axon_start_nrt_profile rc={rc}")
        try:
            yield
        finally:
            n = lib.axon_stop_nrt_profile(str(output_dir).encode())
            if n < 0:
                raise RuntimeError(f"axon_stop_nrt_profile rc={n}")
            print(f"profile: {n} file(s) written to {output_dir}", file=sys.stderr)

    mod = types.ModuleType("antenv.axon_hooks")
    _state = {"hook": _hook}
    mod.get_axon_ntff_profile_hook = lambda: _state["hook"]
    mod.set_axon_ntff_profile_hook = lambda h: _state.__setitem__("hook", h)
    sys.modules["antenv.axon_hooks"] = mod


def _build_program(bpc):
    import concourse.bacc as bacc
    import concourse.mybir as mybir
    import concourse.tile as tile

    f32 = mybir.dt.float32
    f16 = mybir.dt.float16
    bf16 = mybir.dt.bfloat16
    Exp = mybir.ActivationFunctionType.Exp
    Mult = mybir.AluOpType.mult

    nc = bacc.Bacc(
        "TRN2",
        target_bir_lowering=False,
        debug=False,
        enable_asserts=False,
        num_devices=NCORES,
    )
    Q = 4
    assert bpc % Q == 0
    nquads = bpc // Q

    xt_d = nc.dram_tensor("xt", [nquads, P, Q, EC, T], bf16, kind="ExternalInput").ap()
    wkq_d = nc.dram_tensor("wkq", [P, EC, P], bf16, kind="ExternalInput").ap()
    wv_d = nc.dram_tensor("wv", [P, EC, H], bf16, kind="ExternalInput").ap()
    um_d = nc.dram_tensor("um", [P, P], bf16, kind="ExternalInput").ap()
    out_d = nc.dram_tensor(
        "out", [P, nquads, Q, 2, HP1], f16, kind="ExternalOutput"
    ).ap()

    with tile.TileContext(nc) as tc:
        with (
            tc.tile_pool(name="const", bufs=1) as constp,
            tc.tile_pool(name="xin", bufs=6) as xpool,
            tc.tile_pool(name="kq", bufs=7) as kqpool,
            tc.tile_pool(name="psb", bufs=3) as ppool,
            tc.tile_pool(name="osb", bufs=3) as opool,
            tc.tile_pool(name="ps_qk", bufs=2, space="PSUM") as ps_qk,
            tc.tile_pool(name="ps_v", bufs=1, space="PSUM") as ps_v,
            tc.tile_pool(name="ps_s", bufs=2, space="PSUM") as ps_s,
            tc.tile_pool(name="ps_o", bufs=1, space="PSUM") as ps_o,
        ):
            # consts + first input quad in two halves (sync-engine DMA issue
            # costs ~650ns each; order = need order)
            wkq = constp.tile([P, EC, P], bf16)
            nc.sync.dma_start(wkq, wkq_d)
            xt0 = xpool.tile([P, Q, EC, T], bf16, name="xt")
            nc.sync.dma_start(xt0[:, 0:2], xt_d[0, :, 0:2])
            wv = constp.tile([P, EC, H], bf16)
            nc.sync.dma_start(wv, wv_d)
            um = constp.tile([P, P], bf16)
            nc.sync.dma_start(um, um_d)
            nc.sync.dma_start(xt0[:, 2:4], xt_d[0, :, 2:4])
            # persistent v_aug tiles (manual double-buffer) with ones column
            vaugs = []
            for i in range(2):
                va = constp.tile([P, Q, 2, HP1], bf16, name=f"vaug{i}")
                nc.vector.memset(va[:, :, :, H : H + 1], 1.0)
                vaugs.append(va)

            # HAM warmup: ~4us of cold dummy matmuls (no DMA dependency) --
            # must exceed the 3.4us HAM activity window to un-throttle the
            # PE clock before real work arrives.
            wrm = constp.tile([P, 2, T], bf16, name="wrm")
            nc.vector.memset(wrm, 0.0)
            wp = ps_s.tile([P, 2, 4, P], f32, name="s_ps")
            for i in range(10):
                nc.tensor.matmul(
                    wp[:, 0, :, :], wrm[:, 0, 0:P], wrm, start=True, stop=True
                )

            def emit_produce_kq(qd):
                """Input DMA + k/q projection + staging for quad qd."""
                if qd == 0:
                    xt = xt0
                else:
                    xt = xpool.tile([P, Q, EC, T], bf16, name="xt")
                    nc.sync.dma_start(xt, xt_d[qd])

                qks = []
                for pr in range(2):
                    s0 = 2 * pr
                    qk_ps = ps_qk.tile([P, 2, T], f32, name="qk_ps")  # 1 bank
                    for c in range(EC):
                        nc.tensor.matmul(
                            qk_ps,
                            wkq[:, c, :],
                            xt[:, s0 : s0 + 2, c, :],
                            start=(c == 0),
                            stop=(c == EC - 1),
                        )
                    qks.append(qk_ps)
                # partition-packed staging for 2-way row-tiled scores:
                # kk[0:64] = k of batches {0,1}, kk[64:128] = k of {2,3};
                # qq[0:64] = q of {0,1},         qq[64:128] = q of {2,3}.
                kk = kqpool.tile([P, 2, T], bf16, name="kk")
                qq = kqpool.tile([P, 2, T], bf16, name="qq")
                nc.vector.tensor_copy(kk[H:P], qks[1][0:H])  # shift +64
                nc.scalar.copy(kk[0:H], qks[0][0:H])  # aligned
                nc.vector.tensor_copy(qq[0:H], qks[0][H:P])  # shift -64
                nc.scalar.copy(qq[H:P], qks[1][H:P])  # aligned
                return xt, kk, qq

            def emit_produce_v(qd, xt):
                """V projection for quad qd (fills the PE while quad qd-1's
                exp->mask chain drains)."""
                v_ps = ps_v.tile([P, Q, 2, H], f32, name="v_ps")  # 1 bank
                for s in range(Q):
                    for j in range(2):
                        for c in range(EC):
                            nc.tensor.matmul(
                                v_ps[:, s, j, :],
                                xt[:, s, c, j * P : (j + 1) * P],
                                wv[:, c, :],
                                start=(c == 0),
                                stop=(c == EC - 1),
                            )
                v_aug = vaugs[qd % 2]
                nc.vector.tensor_copy(v_aug[:, :, :, 0:H], v_ps)
                return v_aug

            def emit_scores(qd, kk, qq, last=False):
                """Packed scores + exp for quad qd."""
                # p_sb dims: [tk-part, sp, tile, blk, tq]; batch s = 2*tile+sp
                p_sb = ppool.tile([P, 2, 2, 3, P], bf16, name="p_sb")
                for sp in range(2):
                    s_ps = ps_s.tile([P, 2, 4, P], f32, name="s_ps")  # 2 banks
                    nc.tensor.matmul(
                        s_ps[:, 0, 0:2, :],
                        kk[0:H, sp, 0:P],
                        qq[0:H, sp, :],
                        start=True,
                        stop=True,
                        tile_position=(0, 0),
                    )
                    nc.tensor.matmul(
                        s_ps[:, 1, 0:2, :],
                        kk[H:P, sp, 0:P],
                        qq[H:P, sp, :],
                        start=True,
                        stop=True,
                        tile_position=(64, 0),
                    )
                    nc.tensor.matmul(
                        s_ps[:, 0, 2, :],
                        kk[0:H, sp, P:T],
                        qq[0:H, sp, P:T],
                        start=True,
                        stop=True,
                        tile_position=(0, 0),
                    )
                    nc.tensor.matmul(
                        s_ps[:, 1, 2, :],
                        kk[H:P, sp, P:T],
                        qq[H:P, sp, P:T],
                        start=True,
                        stop=True,
                        tile_position=(64, 0),
                    )
                    if last:
                        # serial tail: split exp per tile-half for a shorter
                        # dependency chain into the final out matmuls
                        for tl in range(2):
                            nc.scalar.activation(
                                p_sb[:, sp, tl, :, :],
                                s_ps[:, tl, 0:3, :],
                                Exp,
                                scale=0.125,
                            )
                    else:
                        nc.scalar.activation(
                            p_sb[:, sp, :, :, :],
                            s_ps[:, :, 0:3, :],
                            Exp,
                            scale=0.125,
                        )
                return p_sb

            def emit_out(qd, p_sb, v_aug, last=False):
                """Causal mask + out matmuls + output DMA for quad qd."""
                # per-sp masks so each gates only its own out matmuls; the
                # 0::2 stride covers diagonal blocks 0 and 2
                for sp in range(2):
                    eng = nc.vector if last else nc.gpsimd
                    eng.tensor_tensor(
                        p_sb[:, sp, :, 0::2, :],
                        p_sb[:, sp, :, 0::2, :],
                        um[:, None, None, :].to_broadcast([P, 2, 2, P]),
                        Mult,
                    )

                o_sb = opool.tile([P, Q, 2, HP1], f16, name="o_sb")
                for sp in range(2):
                    o_ps = ps_o.tile([P, 2, 2, HP1], f32, name="o_ps")  # 1 bank
                    for tl in range(2):
                        s = 2 * tl + sp
                        nc.tensor.matmul(
                            o_ps[:, tl, 0, :],
                            p_sb[:, sp, tl, 0, :],
                            v_aug[:, s, 0, :],
                            start=True,
                            stop=True,
                        )
                        nc.tensor.matmul(
                            o_ps[:, tl, 1, :],
                            p_sb[:, sp, tl, 1, :],
                            v_aug[:, s, 0, :],
                            start=True,
                            stop=False,
                        )
                        nc.tensor.matmul(
                            o_ps[:, tl, 1, :],
                            p_sb[:, sp, tl, 2, :],
                            v_aug[:, s, 1, :],
                            start=False,
                            stop=True,
                        )
                    # batches of this sp-group are s = sp, sp+2 -> strided dest
                    nc.vector.tensor_copy(o_sb[:, sp :: 2, :, :], o_ps)

                nc.sync.dma_start(out_d[:, qd], o_sb)

            # software pipeline: produce runs one quad ahead of consume (the
            # Tile scheduler interleaves produce(q+1) into consume(q)'s
            # exp->mask latency on its own)
            xt_c, kk_c, qq_c = emit_produce_kq(0)
            v_aug_c = emit_produce_v(0, xt_c)
            for qd in range(nquads):
                last = qd + 1 == nquads
                tail = qd + 2 >= nquads
                if not last:
                    xt_n, kk_n, qq_n = emit_produce_kq(qd + 1)
                    v_aug_n = emit_produce_v(qd + 1, xt_n)
                p_sb = emit_scores(qd, kk_c, qq_c, last=last)
                emit_out(qd, p_sb, v_aug_c, last=tail)
                if not last:
                    kk_c, qq_c, v_aug_c = kk_n, qq_n, v_aug_n

    nc.compile()
    return nc


def _prep_inputs(x, Wq, Wk, Wv, bpc):
    bf = ml_dtypes.bfloat16
    nb = NCORES * bpc
    nq = bpc // 4
    x = np.asarray(x, dtype=np.float32)[:nb]
    # [b, t, e] -> per core [qd, p, s, c, t] with b = qd*4+s, e = c*128+p
    xt = np.ascontiguousarray(
        x.reshape(NCORES, nq, 4, T, EC, P).transpose(0, 1, 5, 2, 4, 3)
    ).astype(bf)
    wkq = np.concatenate(
        [np.asarray(Wk, np.float32), np.asarray(Wq, np.float32)], axis=1
    )  # [E, 128]: k^T on PSUM partitions 0:64, q^T on 64:128
    wkq = np.ascontiguousarray(wkq.reshape(EC, P, P).transpose(1, 0, 2)).astype(bf)
    wv = np.ascontiguousarray(
        np.asarray(Wv, np.float32).reshape(EC, P, H).transpose(1, 0, 2)
    ).astype(bf)
    tril01 = (np.arange(P)[:, None] <= np.arange(P)[None, :]).astype(np.float32)
    um = tril01.astype(bf)
    per_core = []
    for c in range(NCORES):
        per_core.append(
            {
                "xt": xt[c],
                "wkq": wkq,
                "wv": wv,
                "um": um,
            }
        )
    return per_core


def kernel(x, Wq, Wk, Wv, _trace=False, _bpc=BPC):
    """Full inputs in, full output out. Shards batch dim over 8 NeuronCores."""
    from concourse import bass_utils

    if _trace:
        _install_ntff_hook()

    key = ("prog", _bpc)
    if key not in _cache:
        _cache[key] = _build_program(_bpc)
    nc = _cache[key]

    in_maps = _prep_inputs(x, Wq, Wk, Wv, _bpc)
    res = bass_utils.run_bass_kernel_spmd(
        nc, in_maps, core_ids=list(range(NCORES)), trace=_trace
    )
    _cache["last_result"] = res
    nq = _bpc // 4
    # device layout [p, qd, s, j, h] -> [b, t, h] with b=qd*4+s, t=j*128+p;
    # col 64 is the softmax denominator -> divide here
    outs = []
    for r in res.results:
        o = r["out"].astype(np.float32)
        o = o.reshape(P, nq, 4, 2, HP1).transpose(1, 2, 3, 0, 4)
        o = np.ascontiguousarray(o).reshape(_bpc, T, HP1)
        outs.append(o[:, :, 0:H] / o[:, :, H : H + 1])
    out = np.concatenate(outs, axis=0)
    return out.astype(np.float32)


# revision 9
# speedup vs baseline: 1.2080x; 1.0770x over previous
"""Trainium2 Bass kernel: batched causal single-head self-attention.

Reference computation (per batch b):
    q = x @ Wq; k = x @ Wk; v = x @ Wv          # [T, H] each, contraction over E
    S = (q @ k^T) / sqrt(H)                     # [T, T]
    P = softmax(causal_mask(S), axis=-1)
    out = P @ v                                 # [T, H]

Shapes: x [512, 256, 384] f32, W* [384, 64] f32, out [512, 256, 64] f32.
Sharding: pure data parallel, 64 batches per NeuronCore across 8 cores.

Device algorithm, quad-granular (4 batches per step), matmuls bf16,
software-pipelined so quad i+1's projections overlap quad i's attention:
  - host ships x^T quad-contiguous ([qd, p, s, c, t]): input DMA = 128
    descriptors x 6KB contiguous (first quad split in two for faster ramp).
  - [k^T; q^T] = [Wk|Wq]^T @ xT  (packed 128-wide stationary, 3 E-chunks,
    512-col moving, one PSUM tile per batch-pair A={0,1}, B={2,3}).
  - k/q staged to SBUF in a PARTITION-PACKED layout for 2-way row-tiled
    scores: kk[0:64]=k(A) / kk[64:128]=k(B), qq[0:64]=q(A) / qq[64:128]=q(B).
    Aligned halves copied on ScalarE, cross-half copies on VectorE (the DVE
    output crossbar supports 64-channel partition-shifted writes).
  - v computed directly in [t, h] layout: stationary = x^T chunk (FWL makes
    the 24 small LDWs cheap), moving = Wv chunk. No PE transposes.  The v
    matmuls are emitted BETWEEN quad q's scores and outs so they fill the
    PE while the exp->mask chain drains.
  - v_aug: persistent SBUF tiles with a ones column at h=64 -> the out
    matmul's column 64 is the softmax denominator for free.
  - scores: K=64 matmuls PACKED 2-per-slot via tile_position=(0,0)/(64,0) --
    batches s and s+2 run CONCURRENTLY in the two 64-row halves of the PE.
  - P    = exp(0.125 * S^T)      (ScalarE, per si-pack; no max-subtraction
    needed, |s| < ~45)
  - P   *= causal 0/1 mask       (diagonal blocks only; per-sp on GpSimdE so
    each gates only its own out matmuls; VectorE near the serial tail)
  - out_aug[tq, 0:65] = P^T V_aug; col 64 = denominator; divided on HOST.
    Output DMA'd in device layout [p, qd, s, j, h] as FP16 (halves output
    HBM traffic; numerator/denominator both fit fp16 comfortably).
  - 6 dummy 512-col matmuls at kernel start trip the PE HAM clock gate
    while the first input DMAs are still landing.
"""

import numpy as np
import ml_dtypes

B, T, E, H = 512, 256, 384, 64
NCORES = 8
BPC = B // NCORES  # 64
P = 128
EC = E // P  # 3
HP1 = H + 1  # 65

_cache: dict = {}


def _install_ntff_hook():
    """Shim antenv.axon_hooks (absent in this image) so run_bass_kernel_spmd
    trace=True can capture NTFF profiles via the axon .so's C ABI."""
    import contextlib
    import ctypes
    import sys
    import types

    if "antenv.axon_hooks" in sys.modules:
        return
    so_path = "/opt/axon/libaxon_pjrt.so"
    lib = ctypes.CDLL(so_path)
    if not hasattr(lib, "axon_start_nrt_profile"):
        return
    lib.axon_start_nrt_profile.argtypes = [
        ctypes.POINTER(ctypes.c_int64),
        ctypes.c_size_t,
    ]
    lib.axon_start_nrt_profile.restype = ctypes.c_int64
    lib.axon_stop_nrt_profile.argtypes = [ctypes.c_char_p]
    lib.axon_stop_nrt_profile.restype = ctypes.c_int64

    @contextlib.contextmanager
    def _hook(output_dir, device_ids):
        import jax

        jax.devices()
        if device_ids:
            ids = (ctypes.c_int64 * len(device_ids))(*device_ids)
            rc = lib.axon_start_nrt_profile(ids, len(device_ids))
        else:
            rc = lib.axon_start_nrt_profile(None, 0)
        if rc != 0:
            raise RuntimeError(f"axon_start_nrt_profile rc={rc}")
        try:
            yield
        finally:
            n = lib.axon_stop_nrt_profile(str(output_dir).encode())
            if n < 0:
                raise RuntimeError(f"axon_stop_nrt_profile rc={n}")
            print(f"profile: {n} file(s) written to {output_dir}", file=sys.stderr)

    mod = types.ModuleType("antenv.axon_hooks")
    _state = {"hook": _hook}
    mod.get_axon_ntff_profile_hook = lambda: _state["hook"]
    mod.set_axon_ntff_profile_hook = lambda h: _state.__setitem__("hook", h)
    sys.modules["antenv.axon_hooks"] = mod


def _build_program(bpc):
    import concourse.bacc as bacc
    import concourse.mybir as mybir
    import concourse.tile as tile

    f32 = mybir.dt.float32
    f16 = mybir.dt.float16
    bf16 = mybir.dt.bfloat16
    Exp = mybir.ActivationFunctionType.Exp
    Mult = mybir.AluOpType.mult

    nc = bacc.Bacc(
        "TRN2",
        target_bir_lowering=False,
        debug=False,
        enable_asserts=False,
        num_devices=NCORES,
    )
    Q = 4
    assert bpc % Q == 0
    nquads = bpc // Q

    xt_d = nc.dram_tensor("xt", [nquads, P, Q, EC, T], bf16, kind="ExternalInput").ap()
    wkq_d = nc.dram_tensor("wkq", [P, EC, P], bf16, kind="ExternalInput").ap()
    wv_d = nc.dram_tensor("wv", [P, EC, H], bf16, kind="ExternalInput").ap()
    um_d = nc.dram_tensor("um", [P, P], bf16, kind="ExternalInput").ap()
    out_d = nc.dram_tensor(
        "out", [P, nquads, Q, 2, HP1], f16, kind="ExternalOutput"
    ).ap()

    with tile.TileContext(nc) as tc:
        with (
            tc.tile_pool(name="const", bufs=1) as constp,
            tc.tile_pool(name="xin", bufs=6) as xpool,
            tc.tile_pool(name="kq", bufs=7) as kqpool,
            tc.tile_pool(name="psb", bufs=3) as ppool,
            tc.tile_pool(name="osb", bufs=3) as opool,
            tc.tile_pool(name="ps_qk", bufs=2, space="PSUM") as ps_qk,
            tc.tile_pool(name="ps_v", bufs=1, space="PSUM") as ps_v,
            tc.tile_pool(name="ps_s", bufs=2, space="PSUM") as ps_s,
            tc.tile_pool(name="ps_o", bufs=1, space="PSUM") as ps_o,
        ):
            # consts + first input quad in two halves (sync-engine DMA issue
            # costs ~650ns each; order = need order)
            wkq = constp.tile([P, EC, P], bf16)
            nc.sync.dma_start(wkq, wkq_d)
            xt0 = xpool.tile([P, Q, EC, T], bf16, name="xt")
            nc.sync.dma_start(xt0[:, 0:2], xt_d[0, :, 0:2])
            wv = constp.tile([P, EC, H], bf16)
            nc.sync.dma_start(wv, wv_d)
            um = constp.tile([P, P], bf16)
            nc.sync.dma_start(um, um_d)
            nc.sync.dma_start(xt0[:, 2:4], xt_d[0, :, 2:4])
            # persistent v_aug tiles (manual double-buffer) with ones column
            vaugs = []
            for i in range(2):
                va = constp.tile([P, Q, 2, HP1], bf16, name=f"vaug{i}")
                nc.vector.memset(va[:, :, :, H : H + 1], 1.0)
                vaugs.append(va)

            # HAM warmup: ~4us of cold dummy matmuls (no DMA dependency) --
            # must exceed the 3.4us HAM activity window to un-throttle the
            # PE clock before real work arrives.
            wrm = constp.tile([P, 2, T], bf16, name="wrm")
            nc.vector.memset(wrm, 0.0)
            wp = ps_s.tile([P, 2, 4, P], f32, name="s_ps")
            for i in range(10):
                nc.tensor.matmul(
                    wp[:, 0, :, :], wrm[:, 0, 0:P], wrm, start=True, stop=True
                )

            def emit_produce_kq(qd):
                """Input DMA + k/q projection + staging for quad qd."""
                if qd == 0:
                    xt = xt0
                elif qd <= 2:
                    # ramp: pair-granular DMA so pair-A matmuls start sooner
                    xt = xpool.tile([P, Q, EC, T], bf16, name="xt")
                    nc.sync.dma_start(xt[:, 0:2], xt_d[qd, :, 0:2])
                    nc.sync.dma_start(xt[:, 2:4], xt_d[qd, :, 2:4])
                else:
                    xt = xpool.tile([P, Q, EC, T], bf16, name="xt")
                    nc.sync.dma_start(xt, xt_d[qd])

                qks = []
                for pr in range(2):
                    s0 = 2 * pr
                    qk_ps = ps_qk.tile([P, 2, T], f32, name="qk_ps")  # 1 bank
                    for c in range(EC):
                        nc.tensor.matmul(
                            qk_ps,
                            wkq[:, c, :],
                            xt[:, s0 : s0 + 2, c, :],
                            start=(c == 0),
                            stop=(c == EC - 1),
                        )
                    qks.append(qk_ps)
                # partition-packed staging for 2-way row-tiled scores:
                # kk[0:64] = k of batches {0,1}, kk[64:128] = k of {2,3};
                # qq[0:64] = q of {0,1},         qq[64:128] = q of {2,3}.
                kk = kqpool.tile([P, 2, T], bf16, name="kk")
                qq = kqpool.tile([P, 2, T], bf16, name="qq")
                nc.vector.tensor_copy(kk[H:P], qks[1][0:H])  # shift +64
                nc.scalar.copy(kk[0:H], qks[0][0:H])  # aligned
                nc.vector.tensor_copy(qq[0:H], qks[0][H:P])  # shift -64
                nc.scalar.copy(qq[H:P], qks[1][H:P])  # aligned
                return xt, kk, qq

            def emit_produce_v(qd, xt):
                """V projection for quad qd (fills the PE while quad qd-1's
                exp->mask chain drains)."""
                v_ps = ps_v.tile([P, Q, 2, H], f32, name="v_ps")  # 1 bank
                for s in range(Q):
                    for j in range(2):
                        for c in range(EC):
                            nc.tensor.matmul(
                                v_ps[:, s, j, :],
                                xt[:, s, c, j * P : (j + 1) * P],
                                wv[:, c, :],
                                start=(c == 0),
                                stop=(c == EC - 1),
                            )
                v_aug = vaugs[qd % 2]
                nc.vector.tensor_copy(v_aug[:, :, :, 0:H], v_ps)
                return v_aug

            def emit_scores(qd, kk, qq, last=False):
                """Packed scores + exp for quad qd."""
                # p_sb dims: [tk-part, sp, tile, blk, tq]; batch s = 2*tile+sp
                p_sb = ppool.tile([P, 2, 2, 3, P], bf16, name="p_sb")
                for sp in range(2):
                    s_ps = ps_s.tile([P, 2, 4, P], f32, name="s_ps")  # 2 banks
                    nc.tensor.matmul(
                        s_ps[:, 0, 0:2, :],
                        kk[0:H, sp, 0:P],
                        qq[0:H, sp, :],
                        start=True,
                        stop=True,
                        tile_position=(0, 0),
                    )
                    nc.tensor.matmul(
                        s_ps[:, 1, 0:2, :],
                        kk[H:P, sp, 0:P],
                        qq[H:P, sp, :],
                        start=True,
                        stop=True,
                        tile_position=(64, 0),
                    )
                    nc.tensor.matmul(
                        s_ps[:, 0, 2, :],
                        kk[0:H, sp, P:T],
                        qq[0:H, sp, P:T],
                        start=True,
                        stop=True,
                        tile_position=(0, 0),
                    )
                    nc.tensor.matmul(
                        s_ps[:, 1, 2, :],
                        kk[H:P, sp, P:T],
                        qq[H:P, sp, P:T],
                        start=True,
                        stop=True,
                        tile_position=(64, 0),
                    )
                    if last:
                        # serial tail: split exp per tile-half for a shorter
                        # dependency chain into the final out matmuls
                        for tl in range(2):
                            nc.scalar.activation(
                                p_sb[:, sp, tl, :, :],
                                s_ps[:, tl, 0:3, :],
                                Exp,
                                scale=0.125,
                            )
                    else:
                        nc.scalar.activation(
                            p_sb[:, sp, :, :, :],
                            s_ps[:, :, 0:3, :],
                            Exp,
                            scale=0.125,
                        )
                return p_sb

            def emit_out(qd, p_sb, v_aug, last=False):
                """Causal mask + out matmuls + output DMA for quad qd."""
                # per-sp masks so each gates only its own out matmuls; the
                # 0::2 stride covers diagonal blocks 0 and 2.  sp0's mask is
                # the critical one (gates the first out group + its LDW
                # pull-ahead) -> fast VectorE; sp1's outs have filler -> the
                # otherwise-idle GpSimdE.
                for sp in range(2):
                    eng = nc.vector if (last or sp == 0) else nc.gpsimd
                    eng.tensor_tensor(
                        p_sb[:, sp, :, 0::2, :],
                        p_sb[:, sp, :, 0::2, :],
                        um[:, None, None, :].to_broadcast([P, 2, 2, P]),
                        Mult,
                    )

                o_sb = opool.tile([P, Q, 2, HP1], f16, name="o_sb")
                for sp in range(2):
                    o_ps = ps_o.tile([P, 2, 2, HP1], f32, name="o_ps")  # 1 bank
                    for tl in range(2):
                        s = 2 * tl + sp
                        nc.tensor.matmul(
                            o_ps[:, tl, 0, :],
                            p_sb[:, sp, tl, 0, :],
                            v_aug[:, s, 0, :],
                            start=True,
                            stop=True,
                        )
                        nc.tensor.matmul(
                            o_ps[:, tl, 1, :],
                            p_sb[:, sp, tl, 1, :],
                            v_aug[:, s, 0, :],
                            start=True,
                            stop=False,
                        )
                        nc.tensor.matmul(
                            o_ps[:, tl, 1, :],
                            p_sb[:, sp, tl, 2, :],
                            v_aug[:, s, 1, :],
                            start=False,
                            stop=True,
                        )
                    # batches of this sp-group are s = sp, sp+2 -> strided dest
                    nc.vector.tensor_copy(o_sb[:, sp :: 2, :, :], o_ps)

                nc.sync.dma_start(out_d[:, qd], o_sb)

            # software pipeline: produce runs one quad ahead of consume (the
            # Tile scheduler interleaves produce(q+1) into consume(q)'s
            # exp->mask latency on its own)
            xt_c, kk_c, qq_c = emit_produce_kq(0)
            v_aug_c = emit_produce_v(0, xt_c)
            for qd in range(nquads):
                last = qd + 1 == nquads
                tail = qd + 2 >= nquads
                if not last:
                    xt_n, kk_n, qq_n = emit_produce_kq(qd + 1)
                    v_aug_n = emit_produce_v(qd + 1, xt_n)
                p_sb = emit_scores(qd, kk_c, qq_c, last=last)
                emit_out(qd, p_sb, v_aug_c, last=tail)
                if not last:
                    kk_c, qq_c, v_aug_c = kk_n, qq_n, v_aug_n

    nc.compile()
    return nc


def _prep_inputs(x, Wq, Wk, Wv, bpc):
    bf = ml_dtypes.bfloat16
    nb = NCORES * bpc
    nq = bpc // 4
    x = np.asarray(x, dtype=np.float32)[:nb]
    # [b, t, e] -> per core [qd, p, s, c, t] with b = qd*4+s, e = c*128+p
    xt = np.ascontiguousarray(
        x.reshape(NCORES, nq, 4, T, EC, P).transpose(0, 1, 5, 2, 4, 3)
    ).astype(bf)
    wkq = np.concatenate(
        [np.asarray(Wk, np.float32), np.asarray(Wq, np.float32)], axis=1
    )  # [E, 128]: k^T on PSUM partitions 0:64, q^T on 64:128
    wkq = np.ascontiguousarray(wkq.reshape(EC, P, P).transpose(1, 0, 2)).astype(bf)
    wv = np.ascontiguousarray(
        np.asarray(Wv, np.float32).reshape(EC, P, H).transpose(1, 0, 2)
    ).astype(bf)
    tril01 = (np.arange(P)[:, None] <= np.arange(P)[None, :]).astype(np.float32)
    um = tril01.astype(bf)
    per_core = []
    for c in range(NCORES):
        per_core.append(
            {
                "xt": xt[c],
                "wkq": wkq,
                "wv": wv,
                "um": um,
            }
        )
    return per_core


def kernel(x, Wq, Wk, Wv, _trace=False, _bpc=BPC):
    """Full inputs in, full output out. Shards batch dim over 8 NeuronCores."""
    from concourse import bass_utils

    if _trace:
        _install_ntff_hook()

    key = ("prog", _bpc)
    if key not in _cache:
        _cache[key] = _build_program(_bpc)
    nc = _cache[key]

    in_maps = _prep_inputs(x, Wq, Wk, Wv, _bpc)
    res = bass_utils.run_bass_kernel_spmd(
        nc, in_maps, core_ids=list(range(NCORES)), trace=_trace
    )
    _cache["last_result"] = res
    nq = _bpc // 4
    # device layout [p, qd, s, j, h] -> [b, t, h] with b=qd*4+s, t=j*128+p;
    # col 64 is the softmax denominator -> divide here
    outs = []
    for r in res.results:
        o = r["out"].astype(np.float32)
        o = o.reshape(P, nq, 4, 2, HP1).transpose(1, 2, 3, 0, 4)
        o = np.ascontiguousarray(o).reshape(_bpc, T, HP1)
        outs.append(o[:, :, 0:H] / o[:, :, H : H + 1])
    out = np.concatenate(outs, axis=0)
    return out.astype(np.float32)
